# revision 5
# baseline (speedup 1.0000x reference)
"""HashEncoding (instant-NGP style) TRN2 Bass kernel.

Data-parallel over 8 NeuronCores: each core processes N/8 = 131072 points and
produces its slice of the [N, 32] output. Hash tables are replicated per core
in device DRAM as two derived tables built host-side:

* ``tbl_l``: for the 5 non-hashed (dense) levels, a row-expanded table with
  2*size rows of 16 floats: row v holds all 8 corner feature pairs
  ``T[(v + dx + dy*s + dz*s^2) % size]`` so one 64-byte gather per point per
  level fetches every corner (and absorbs the reference's ``% size`` wrap).
* ``tbl_h``: the 11 hashed levels concatenated ([11*2^19, 2] f32); the spatial
  hash is computed on-device with exact fp32/bitwise arithmetic.

Gathers use the (HW-verified) one-index-per-partition indirect DMA: each
instruction gathers 128 rows (one per SBUF partition). Index computation,
trilinear weights and the blend run on the vector engine in fp32 with
rounding matched to the reference (mul-then-add, floor via the 2^23 trick).
"""

import math

import numpy as np

import concourse.bacc as bacc
import concourse.mybir as mybir
from concourse import bass_utils
from concourse.bass import IndirectOffsetOnAxis, ds
from concourse.tile import TileContext

# ---------------------------------------------------------------- constants
N_LEVELS = 16
N_FEAT = 2
LOG2_HASHMAP = 19
BASE_RES = 16
PER_LEVEL_SCALE = 1.38191288
N_POINTS = 1 << 20
N_CORES = 8
NPC = N_POINTS // N_CORES  # points per core

MASK = (1 << LOG2_HASHMAP) - 1
C1P = 2654435761 % (1 << LOG2_HASHMAP)  # 489905
C2P = 805459861 % (1 << LOG2_HASHMAP)  # 153493
A1, B1 = C1P // 512, C1P % 512  # 956, 433
A2, B2 = C2P // 512, C2P % 512  # 299, 405

OP = mybir.AluOpType


def _level_params():
    params = []
    for i in range(N_LEVELS):
        scale = math.pow(2.0, i * math.log2(PER_LEVEL_SCALE)) * BASE_RES - 1.0
        res = math.ceil(scale) + 1
        size = min(math.ceil(res**3 / 8) * 8, 1 << LOG2_HASHMAP)
        hashed = size >= (1 << LOG2_HASHMAP)
        params.append((scale, size, 0 if hashed else res))
    return params


LEVEL_PARAMS = _level_params()
LIN_LEVELS = [l for l, p in enumerate(LEVEL_PARAMS) if p[2] != 0]  # 0..4
HASH_LEVELS = [l for l, p in enumerate(LEVEL_PARAMS) if p[2] == 0]  # 5..15
NL = len(LIN_LEVELS)  # 5
NH = len(HASH_LEVELS)  # 11

LIN_BASES = {}
_off = 0
for _l in LIN_LEVELS:
    LIN_BASES[_l] = _off
    _off += 2 * LEVEL_PARAMS[_l][1]
LIN_ROWS = _off
HASH_BASES = {l: (i << LOG2_HASHMAP) for i, l in enumerate(HASH_LEVELS)}
HASH_ROWS = NH << LOG2_HASHMAP

# tiling
T = 64  # point columns per partition per tile
PTS_PER_TILE = 128 * T  # 8192
NT = NPC // PTS_PER_TILE  # 16 tiles per core
GH_PER_PT = 8 * NH  # 88 hashed gathers per point
GL_PER_PT = NL  # 5 linear gathers per point
UNROLL = 16  # gathers per inner-loop iteration

F32 = mybir.dt.float32
I32 = mybir.dt.int32
BIG = float(2.0**23)


# ---------------------------------------------------------------- host tables
def build_tables(tables):
    """tables: tuple of 16 [size_l, 2] f32 arrays -> (tbl_l, tbl_h)."""
    lin_parts = []
    for l in LIN_LEVELS:
        tab = np.asarray(tables[l])
        size = LEVEL_PARAMS[l][1]
        res = LEVEL_PARAMS[l][2]
        v = np.arange(2 * size, dtype=np.int64)
        cols = []
        for c in range(8):
            xb, yb, zb = (c >> 2) & 1, (c >> 1) & 1, c & 1
            off = xb + yb * res + zb * res * res
            cols.append(tab[(v + off) % size])  # [2*size, 2]
        lin_parts.append(np.concatenate(cols, axis=1))  # [2*size, 16]
    tbl_l = np.ascontiguousarray(np.concatenate(lin_parts, axis=0), np.float32)
    tbl_h = np.ascontiguousarray(
        np.concatenate([np.asarray(tables[l]) for l in HASH_LEVELS], axis=0),
        np.float32,
    )
    assert tbl_l.shape == (LIN_ROWS, 16)
    assert tbl_h.shape == (HASH_ROWS, 2)
    return tbl_l, tbl_h


# ---------------------------------------------------------------- device build
def build_kernel(nt=NT):
    nc = bacc.Bacc("TRN2", target_bir_lowering=False, debug=False, num_devices=1)
    pts = nc.dram_tensor("pts", [nt * PTS_PER_TILE, 3], F32, kind="ExternalInput")
    tbl_l = nc.dram_tensor("tbl_l", [LIN_ROWS, 16], F32, kind="ExternalInput")
    tbl_h = nc.dram_tensor("tbl_h", [HASH_ROWS, 2], F32, kind="ExternalInput")
    out = nc.dram_tensor("out", [nt * PTS_PER_TILE, 32], F32, kind="ExternalOutput")

    def ts(out_ap, in_ap, scalar, op):
        nc.vector.tensor_scalar(out=out_ap, in0=in_ap, scalar1=scalar,
                                scalar2=None, op0=op)

    def tt(out_ap, a_ap, b_ap, op):
        nc.vector.tensor_tensor(out=out_ap, in0=a_ap, in1=b_ap, op=op)

    with TileContext(nc) as tc:
        with tc.tile_pool(name="sbuf", bufs=1) as pool:
            pts_t = pool.tile([128, T * 3], F32, tag="pts")
            ih = pool.tile([128, T * GH_PER_PT], I32, tag="ih")
            il = pool.tile([128, T * GL_PER_PT], I32, tag="il")
            gh = pool.tile([128, T * GH_PER_PT * 2], F32, tag="gh")
            gl = pool.tile([128, T * GL_PER_PT * 16], F32, tag="gl")
            acc = pool.tile([128, T * 32], F32, tag="acc")
            wbig = pool.tile([128, T * N_LEVELS * 8], F32, tag="wbig")
            w8 = [pool.tile([128, T], F32, name=f"w{c}", tag=f"w{c}") for c in range(8)]
            sc = {k: pool.tile([128, T], F32, name=f"sc_{k}", tag=f"sc_{k}") for k in
                  ("pos", "f", "gt", "gx", "gy", "gz", "dx", "dy", "dz",
                   "q", "t3")}
            ii = {k: pool.tile([128, T], I32, name=f"ii_{k}", tag=f"ii_{k}") for k in
                  ("xi", "x1", "hy0", "hy1", "hz0", "hz1", "a", "t")}
            sidx = pool.tile([128, UNROLL], I32, tag="sidx")
            sgat_h = pool.tile([128, UNROLL * 2], F32, tag="sgath")
            sgat_l = pool.tile([128, UNROLL * 16], F32, tag="sgatl")

            ih_g = ih[:].rearrange("p (u g) -> p g u", g=GH_PER_PT)
            il_g = il[:].rearrange("p (u g) -> p g u", g=GL_PER_PT)
            gh_g = gh[:].rearrange("p (u e) -> p e u", e=GH_PER_PT * 2)
            gl_g = gl[:].rearrange("p (u e) -> p e u", e=GL_PER_PT * 16)
            acc_g = acc[:].rearrange("p (u e) -> p e u", e=32)
            w_g = wbig[:].rearrange("p (u l c) -> p l c u", l=N_LEVELS, c=8)

            def floordelta(src_ap, scale, g_t, d_t):
                """g_t = floor(f32(f32(src*scale) + 0.5)); d_t = pos - g_t."""
                ts(sc["pos"][:], src_ap, float(np.float32(scale)), OP.mult)
                ts(sc["pos"][:], sc["pos"][:], 0.5, OP.add)
                ts(sc["f"][:], sc["pos"][:], BIG, OP.add)
                ts(sc["f"][:], sc["f"][:], BIG, OP.subtract)
                tt(sc["gt"][:], sc["f"][:], sc["pos"][:], OP.is_gt)
                tt(g_t[:], sc["f"][:], sc["gt"][:], OP.subtract)
                tt(d_t[:], sc["pos"][:], g_t[:], OP.subtract)

            def hash_prod(y_t, a, b, h_t):
                """h_t = low-19-bit-exact rep of y*(a*512+b); h_t < 2^21."""
                ts(ii["t"][:], y_t[:], a, OP.mult)
                ts(ii["t"][:], ii["t"][:], 1023, OP.bitwise_and)
                ts(ii["t"][:], ii["t"][:], 9, OP.logical_shift_left)
                ts(h_t[:], y_t[:], b, OP.mult)
                tt(h_t[:], h_t[:], ii["t"][:], OP.add)

            def body(t_iv):
                src = pts[ds(t_iv * PTS_PER_TILE, PTS_PER_TILE)].rearrange(
                    "(p u) c -> p (u c)", p=128)
                nc.sync.dma_start(pts_t[:], src)
                coords = pts_t[:].rearrange("p (u c) -> p c u", c=3)
                xs, ys, zs = coords[:, 0], coords[:, 1], coords[:, 2]

                for lvl in range(N_LEVELS):
                    scale, size, res = LEVEL_PARAMS[lvl]
                    floordelta(xs, scale, sc["gx"], sc["dx"])
                    floordelta(ys, scale, sc["gy"], sc["dy"])
                    floordelta(zs, scale, sc["gz"], sc["dz"])

                    # weights (c = xb*4 + yb*2 + zb):
                    # p11 = dy*dz; p01 = dz - p11; p10 = dy - p11;
                    # p00 = (1-dy) - p01
                    tt(w8[3][:], sc["dy"][:], sc["dz"][:], OP.mult)   # yz=11
                    tt(w8[1][:], sc["dz"][:], w8[3][:], OP.subtract)  # yz=01
                    tt(w8[2][:], sc["dy"][:], w8[3][:], OP.subtract)  # yz=10
                    ts(sc["q"][:], sc["dy"][:], -1.0, OP.mult)
                    ts(sc["q"][:], sc["q"][:], 1.0, OP.add)           # 1-dy
                    tt(w8[0][:], sc["q"][:], w8[1][:], OP.subtract)   # yz=00
                    for c in range(4):
                        tt(w8[4 + c][:], w8[c][:], sc["dx"][:], OP.mult)
                        tt(w8[c][:], w8[c][:], w8[4 + c][:], OP.subtract)
                    for c in range(8):
                        nc.vector.tensor_copy(out=w_g[:, lvl, c], in_=w8[c][:])

                    if res == 0:
                        hidx = HASH_LEVELS.index(lvl)
                        base = HASH_BASES[lvl]
                        nc.vector.tensor_copy(out=ii["xi"][:], in_=sc["gx"][:])
                        ts(ii["x1"][:], ii["xi"][:], 1, OP.add)
                        nc.vector.tensor_copy(out=ii["hy0"][:], in_=sc["gy"][:])
                        hash_prod(ii["hy0"], A1, B1, ii["hy0"])
                        ts(ii["hy1"][:], ii["hy0"][:], C1P, OP.add)
                        nc.vector.tensor_copy(out=ii["hz0"][:], in_=sc["gz"][:])
                        hash_prod(ii["hz0"], A2, B2, ii["hz0"])
                        ts(ii["hz1"][:], ii["hz0"][:], C2P, OP.add)
                        for c in range(8):
                            xb, yb, zb = (c >> 2) & 1, (c >> 1) & 1, c & 1
                            hy = ii["hy1"] if yb else ii["hy0"]
                            hz = ii["hz1"] if zb else ii["hz0"]
                            xv = ii["x1"] if xb else ii["xi"]
                            tt(ii["a"][:], hy[:], hz[:], OP.bitwise_xor)
                            tt(ii["a"][:], ii["a"][:], xv[:], OP.bitwise_xor)
                            ts(ii["a"][:], ii["a"][:], MASK, OP.bitwise_and)
                            ts(ih_g[:, hidx * 8 + c], ii["a"][:], base,
                               OP.bitwise_or)
                    else:
                        base = LIN_BASES[lvl]
                        ts(sc["t3"][:], sc["gy"][:], float(res), OP.mult)
                        tt(sc["t3"][:], sc["t3"][:], sc["gx"][:], OP.add)
                        ts(sc["f"][:], sc["gz"][:], float(res * res), OP.mult)
                        tt(sc["t3"][:], sc["t3"][:], sc["f"][:], OP.add)
                        ts(sc["t3"][:], sc["t3"][:], float(base), OP.add)
                        nc.vector.tensor_copy(
                            out=il_g[:, LIN_LEVELS.index(lvl)], in_=sc["t3"][:])

                # gather: hashed (2 f32 per index)
                n_it_h = T * GH_PER_PT // UNROLL
                with tc.For_i(0, n_it_h, 1) as i:
                    nc.vector.tensor_copy(out=sidx[:],
                                          in_=ih[:, ds(i * UNROLL, UNROLL)])
                    for u in range(UNROLL):
                        nc.gpsimd.indirect_dma_start(
                            out=sgat_h[:, u * 2:(u + 1) * 2],
                            out_offset=None,
                            in_=tbl_h[:],
                            in_offset=IndirectOffsetOnAxis(
                                ap=sidx[:, u:u + 1], axis=0),
                        )
                    nc.vector.tensor_copy(
                        out=gh[:, ds(i * (UNROLL * 2), UNROLL * 2)],
                        in_=sgat_h[:])

                # gather: linear (16 f32 per index)
                n_it_l = T * GL_PER_PT // UNROLL
                with tc.For_i(0, n_it_l, 1) as i:
                    nc.vector.tensor_copy(out=sidx[:],
                                          in_=il[:, ds(i * UNROLL, UNROLL)])
                    for u in range(UNROLL):
                        nc.gpsimd.indirect_dma_start(
                            out=sgat_l[:, u * 16:(u + 1) * 16],
                            out_offset=None,
                            in_=tbl_l[:],
                            in_offset=IndirectOffsetOnAxis(
                                ap=sidx[:, u:u + 1], axis=0),
                        )
                    nc.vector.tensor_copy(
                        out=gl[:, ds(i * (UNROLL * 16), UNROLL * 16)],
                        in_=sgat_l[:])

                # blend
                for lvl in range(N_LEVELS):
                    for f in range(2):
                        asl = acc_g[:, lvl * 2 + f]
                        for c in range(8):
                            wv = w_g[:, lvl, c]
                            if LEVEL_PARAMS[lvl][2] == 0:
                                hidx = HASH_LEVELS.index(lvl)
                                gsl = gh_g[:, (hidx * 8 + c) * 2 + f]
                            else:
                                lidx = LIN_LEVELS.index(lvl)
                                gsl = gl_g[:, lidx * 16 + c * 2 + f]
                            if c == 0:
                                tt(asl, gsl, wv, OP.mult)
                            else:
                                tt(sc["t3"][:], gsl, wv, OP.mult)
                                tt(asl, asl, sc["t3"][:], OP.add)

                dst = out[ds(t_iv * PTS_PER_TILE, PTS_PER_TILE)].rearrange(
                    "(p u) e -> p (u e)", p=128)
                nc.sync.dma_start(dst, acc[:])

            with tc.For_i(0, nt, 1) as t_iv:
                body(t_iv)

    nc.compile()
    return nc


_CACHE = {}


def _get_kernel(nt):
    if nt not in _CACHE:
        _CACHE[nt] = build_kernel(nt)
    return _CACHE[nt]


# ---------------------------------------------------------------- entry point
LAST_RESULTS = None
LAST_EXEC_WALL = None


def kernel(inputs, tables):
    global LAST_RESULTS
    inputs = np.ascontiguousarray(np.asarray(inputs), np.float32)
    assert inputs.shape == (N_POINTS, 3)
    tbl_l, tbl_h = build_tables(tables)

    nc = _get_kernel(NT)
    in_maps = []
    for c in range(N_CORES):
        sl = np.ascontiguousarray(inputs[c * NPC:(c + 1) * NPC])
        in_maps.append({"pts": sl, "tbl_l": tbl_l, "tbl_h": tbl_h})
    import time as _time
    global LAST_EXEC_WALL
    _t0 = _time.time()
    res = bass_utils.run_bass_kernel_spmd(nc, in_maps,
                                          core_ids=list(range(N_CORES)))
    LAST_EXEC_WALL = _time.time() - _t0
    LAST_RESULTS = res
    return np.concatenate([res.results[c]["out"] for c in range(N_CORES)], axis=0)
